# revision 27
# baseline (speedup 1.0000x reference)
"""MoCo retrieval kernel (nn_MoCo) for 8 Trainium2 NeuronCores.

Strategy (K-sharded, two launches):
  Launch A (per core c, shard = queue rows [c*8192, (c+1)*8192)):
    - q/k encoder GEMMs (fp32r matmuls, transposed layout), L2 normalize,
      momentum-update of W_k (scale folded into the normalize),
      logits_pos, out_q export.
    - big GEMM (out_q/T) @ queue_shard^T -> logits slice [256, 8192]
      written straight from PSUM to DRAM.
    - chunk-max hierarchy (chunk=16) + per-row top-40 candidate capture
      using DVE max/max_index/match_replace + one indirect-DMA chunk
      gather. Outputs candidate values/chunk-ids/positions.
  Host glue: decodes candidate columns, merges the 8 per-core sorted
    pools and re-ranks them with fp64 dots (verified to reproduce the
    jax fp32 reference ordering bit-exactly), then builds the gather
    indices for the mixing stage from idxs1a/idxs1b/idxs2.
  Launch B (per core, batch shard of 32 queries): indirect-DMA gathers
    of the selected queue rows + mixing/normalize/dot math -> extra
    logits [256, 16] x2.

Self-contained: hardcodes all shapes from the problem spec.
"""

import os
from contextlib import ExitStack

import numpy as np

import concourse.bass as bass
import concourse.mybir as mybir
import concourse.tile as tile
from concourse import bacc
from concourse.bass import IndirectOffsetOnAxis
from concourse.masks import make_identity

# ---- problem constants (from spec) ----
B, IN_DIM, D, K = 256, 2048, 128, 65536
N_HARD, S1, S2 = 32, 16, 16
T, M_EMA, BETA_HARD = 0.07, 0.999, 0.5
START1, START2 = 1024, 2048

NCORES = 8
KSH = K // NCORES            # 8192 queue rows per core
NKT = KSH // 512             # 16 n-tiles of 512 queue rows
CHUNK = 32
NCHUNK = KSH // CHUNK        # 512 chunks per row per core
NCAP = 40                    # candidates captured per (row, core)
NR = NCAP // 8               # max8 rounds
NEGV = -1.0e30

F32 = mybir.dt.float32
F32R = mybir.dt.float32r
U32 = mybir.dt.uint32
AX = mybir.AxisListType
ALU = mybir.AluOpType
AF = mybir.ActivationFunctionType


def _new_nc():
    return bacc.Bacc("TRN2", target_bir_lowering=False, debug=False)


def _bcast_mid(ap, n):
    """[P, F] AP -> [P, n, F] AP broadcasting along a new middle dim."""
    return bass.AP(ap.tensor, ap.offset, [list(ap.ap[0]), [0, n], list(ap.ap[1])])


# --------------------------------------------------------------------------
# Launch A
# --------------------------------------------------------------------------

def _build_launch_a():
    nc = _new_nc()
    q_d = nc.dram_tensor("q", [B, IN_DIM], F32, kind="ExternalInput")
    k_d = nc.dram_tensor("k", [B, IN_DIM], F32, kind="ExternalInput")
    wq_d = nc.dram_tensor("wq", [IN_DIM, D], F32, kind="ExternalInput")
    wk_d = nc.dram_tensor("wk", [IN_DIM, D], F32, kind="ExternalInput")
    qs_d = nc.dram_tensor("qsh", [KSH, D], F32, kind="ExternalInput")

    L_d = nc.dram_tensor("L", [B, KSH], F32, kind="ExternalOutput")
    oq_d = nc.dram_tensor("outqT", [D, B], F32, kind="ExternalOutput")
    pos_d = nc.dram_tensor("pos", [1, B], F32, kind="ExternalOutput")
    cc_d = nc.dram_tensor("candc", [B, NCAP], U32, kind="ExternalOutput")
    cm_d = nc.dram_tensor("candm", [B, NCAP], F32, kind="ExternalOutput")

    with tile.TileContext(nc) as tc, ExitStack() as ctx:
        const = ctx.enter_context(tc.tile_pool(name="const", bufs=1))
        persist = ctx.enter_context(tc.tile_pool(name="persist", bufs=1))

        id_t = const.tile([128, 128], F32)
        make_identity(nc, id_t)
        ones_col = const.tile([128, 1], F32)
        nc.vector.memset(ones_col, 1.0)
        ones_row = const.tile([1, 128], F32)
        nc.vector.memset(ones_row, 1.0)

        # ---- weights ----
        wq_sb = persist.tile([128, 16, 128], F32)
        wk_sb = persist.tile([128, 16, 128], F32)
        wkn_sb = persist.tile([128, 16, 128], F32)
        nc.sync.dma_start(wq_sb, wq_d.ap().rearrange("(c p) d -> p c d", p=128))
        nc.sync.dma_start(wk_sb, wk_d.ap().rearrange("(c p) d -> p c d", p=128))
        # W_k_new/0.999 = W_k + (0.001/0.999) W_q ; the scale cancels in the
        # L2 normalize of out_k.
        nc.scalar.activation(wkn_sb, wq_sb, AF.Copy, scale=(1.0 - M_EMA) / M_EMA)
        nc.vector.tensor_tensor(out=wkn_sb, in0=wkn_sb, in1=wk_sb, op=ALU.add)

        # ---- q/k transposes: qT/kT [128(k-chunk), 16, 256(b)] ----
        qT = persist.tile([128, 16, 256], F32)
        kT = persist.tile([128, 16, 256], F32)
        oqn = persist.tile([128, 256], F32)   # normalized out_q^T
        okn = persist.tile([128, 256], F32)   # normalized out_k^T
        oqs = persist.tile([128, 256], F32)   # oqn * (1/T), big-GEMM lhsT

        with tc.tile_pool(name="early_sb", bufs=2) as esb, \
             tc.tile_pool(name="ps_t", bufs=2, space="PSUM") as psT, \
             tc.tile_pool(name="ps_o", bufs=2, space="PSUM") as psO, \
             tc.tile_pool(name="ps_s", bufs=2, space="PSUM") as psS:
            for src_d, dstT in ((q_d, qT), (k_d, kT)):
                stage = esb.tile([128, 2, 2048], F32, tag="stage")
                nc.sync.dma_start(
                    stage, src_d.ap().rearrange("(mt p) c -> p mt c", p=128)
                )
                for kc2 in range(8):
                    pt = psT.tile([128, 512], F32, tag="pt")
                    for i in range(2):
                        kc = kc2 * 2 + i
                        for mt in range(2):
                            nc.tensor.transpose(
                                pt[:, i * 256 + mt * 128:i * 256 + (mt + 1) * 128],
                                stage[:, mt, kc * 128:(kc + 1) * 128],
                                id_t,
                            )
                    nc.scalar.activation(
                        dstT[:, kc2 * 2:kc2 * 2 + 2, :], pt, AF.Copy
                    )

            # ---- out_q / out_k (transposed layout) + normalize ----
            for srcT, w_sb, on in ((qT, wq_sb, oqn), (kT, wkn_sb, okn)):
                po = psO.tile([128, 256], F32, tag="po")
                for kc in range(16):
                    nc.tensor.matmul(
                        po,
                        lhsT=w_sb[:, kc, :],
                        rhs=srcT[:, kc, :],
                        start=(kc == 0),
                        stop=(kc == 15),
                    )
                o_raw = esb.tile([128, 256], F32, tag="o_raw")
                nc.scalar.activation(o_raw, po, AF.Copy)
                sq = esb.tile([128, 256], F32, tag="sq")
                nc.vector.tensor_tensor(out=sq, in0=o_raw, in1=o_raw, op=ALU.mult)
                pn = psS.tile([1, 256], F32, tag="pn")
                nc.tensor.matmul(
                    pn, lhsT=ones_col, rhs=sq,
                    start=True, stop=True,
                )
                # rsqrt = reciprocal(sqrt(s)) + one Newton step for accuracy
                rt = esb.tile([1, 256], F32, tag="rt")
                nc.scalar.activation(rt, pn, AF.Sqrt)
                inv0 = esb.tile([1, 256], F32, tag="inv0")
                nc.vector.reciprocal(inv0, rt)
                # inv = inv0 * (1.5 - 0.5*s*inv0^2)
                s_sb = esb.tile([1, 256], F32, tag="s_sb")
                nc.vector.tensor_copy(s_sb, pn)
                t0 = esb.tile([1, 256], F32, tag="t0")
                nc.vector.tensor_tensor(out=t0, in0=inv0, in1=inv0, op=ALU.mult)
                nc.vector.tensor_tensor(out=t0, in0=t0, in1=s_sb, op=ALU.mult)
                nc.vector.tensor_scalar(
                    out=t0, in0=t0, scalar1=-0.5, scalar2=1.5,
                    op0=ALU.mult, op1=ALU.add,
                )
                inv1 = esb.tile([1, 256], F32, tag="inv1")
                nc.vector.tensor_tensor(out=inv1, in0=inv0, in1=t0, op=ALU.mult)
                pb = psO.tile([128, 256], F32, tag="pb")
                nc.tensor.matmul(
                    pb, lhsT=ones_row, rhs=inv1,
                    start=True, stop=True,
                )
                nc.vector.tensor_tensor(out=on, in0=o_raw, in1=pb, op=ALU.mult)

            # logits_pos
            tt = esb.tile([128, 256], F32, tag="tt")
            nc.vector.tensor_tensor(out=tt, in0=oqn, in1=okn, op=ALU.mult)
            pp = psS.tile([1, 256], F32, tag="pn")
            nc.tensor.matmul(
                pp, lhsT=ones_col, rhs=tt,
                start=True, stop=True,
            )
            pos_sb = esb.tile([1, 256], F32, tag="pos_sb")
            nc.scalar.activation(pos_sb, pp, AF.Copy, scale=1.0 / T)
            nc.sync.dma_start(pos_d.ap(), pos_sb)

            nc.vector.tensor_scalar_mul(oqs, oqn, 1.0 / T)
            nc.sync.dma_start(oq_d.ap(), oqn)

        # ---- big GEMM over the queue shard ----
        M0 = persist.tile([128, NCHUNK], F32)
        M1 = persist.tile([128, NCHUNK], F32)
        Ms = (M0, M1)

        with tc.tile_pool(name="qloop_sb", bufs=4) as qsb, \
             tc.tile_pool(name="ps_qt", bufs=3, space="PSUM") as psQT, \
             tc.tile_pool(name="ps_l", bufs=5, space="PSUM") as psL:
            for nt in range(NKT):
                qst = qsb.tile([128, 4, 128], F32, tag="qst")
                nc.sync.dma_start(
                    qst,
                    qs_d.ap()[nt * 512:(nt + 1) * 512, :]
                    .rearrange("(s p) d -> p s d", p=128),
                )
                qt = psQT.tile([128, 512], F32, tag="qt")
                for s in range(4):
                    nc.tensor.transpose(
                        qt[:, s * 128:(s + 1) * 128], qst[:, s, :], id_t
                    )
                qts = qsb.tile([128, 512], F32, tag="qts")
                nc.scalar.activation(qts, qt, AF.Copy)
                for mt in range(2):
                    pl = psL.tile([128, 512], F32, tag="pl")
                    nc.tensor.matmul(
                        pl,
                        lhsT=oqs[:, mt * 128:(mt + 1) * 128],
                        rhs=qts,
                        start=True, stop=True,
                    )
                    lsb = qsb.tile([128, 512], F32, tag="lsb")
                    # split PSUM eviction across ACT and DVE for balance
                    if mt == 0:
                        nc.scalar.activation(lsb, pl, AF.Copy)
                    else:
                        nc.vector.tensor_copy(lsb, pl)
                    nc.vector.tensor_reduce(
                        out=Ms[mt][:, nt * (512 // CHUNK):(nt + 1) * (512 // CHUNK)],
                        in_=lsb.rearrange("p (c s) -> p c s", s=CHUNK),
                        axis=AX.X, op=ALU.max,
                    )
                    nc.sync.dma_start(
                        L_d.ap()[mt * 128:(mt + 1) * 128,
                                 nt * 512:(nt + 1) * 512],
                        lsb,
                    )

        # ---- top-NCAP chunk capture (ids + maxima); host re-ranks contents ----
        with tc.tile_pool(name="topk_sb", bufs=2) as tsb:
            for mt in range(2):
                cc_sb = tsb.tile([128, NCAP], U32, tag="cc_sb")
                cm_sb = tsb.tile([128, NCAP], F32, tag="cm_sb")
                for r in range(NR):
                    v8 = cm_sb[:, r * 8:(r + 1) * 8]
                    nc.vector.max(out=v8, in_=Ms[mt])
                    nc.vector.max_index(
                        out=cc_sb[:, r * 8:(r + 1) * 8],
                        in_max=v8, in_values=Ms[mt],
                    )
                    nc.vector.match_replace(
                        out=Ms[mt], in_to_replace=v8, in_values=Ms[mt],
                        imm_value=NEGV,
                    )
                rows = slice(mt * 128, (mt + 1) * 128)
                nc.sync.dma_start(cc_d.ap()[rows, :], cc_sb)
                nc.sync.dma_start(cm_d.ap()[rows, :], cm_sb)

    nc.compile()
    out_names = ["L", "outqT", "pos", "candc", "candm"]
    return nc, out_names


# --------------------------------------------------------------------------
# Launch B  (mixing of hard negatives; batch-sharded, 32 queries/core)
# --------------------------------------------------------------------------

TAB_B = 3 * 32 * S1  # compact per-core queue-row table for launch B


def _build_launch_b():
    nc = _new_nc()
    queue_d = nc.dram_tensor("tab", [TAB_B, D], F32, kind="ExternalInput")
    oq_d = nc.dram_tensor("oq", [32, D], F32, kind="ExternalInput")
    i1a_d = nc.dram_tensor("i1a", [128, 4], U32, kind="ExternalInput")
    i1b_d = nc.dram_tensor("i1b", [128, 4], U32, kind="ExternalInput")
    i2_d = nc.dram_tensor("i2", [128, 4], U32, kind="ExternalInput")
    a_d = nc.dram_tensor("alpha", [128, 4], F32, kind="ExternalInput")
    bb_d = nc.dram_tensor("bhalf", [128, 4], F32, kind="ExternalInput")
    e1_d = nc.dram_tensor("e1", [128, 4], F32, kind="ExternalOutput")
    e2_d = nc.dram_tensor("e2", [128, 4], F32, kind="ExternalOutput")

    with tile.TileContext(nc) as tc, ExitStack() as ctx:
        sb = ctx.enter_context(tc.tile_pool(name="sb", bufs=1))

        # out_q rows replicated x4 along partitions: partition p -> row p%32
        oq4 = sb.tile([128, 128], F32)
        for g in range(4):
            nc.sync.dma_start(oq4[g * 32:(g + 1) * 32, :], oq_d.ap())
        oq_rep = _bcast_mid(oq4[:, :], 4)          # [128, 4, 128]

        gat = {}
        for name, idx_d in (("g1a", i1a_d), ("g1b", i1b_d), ("g2", i2_d)):
            isb = sb.tile([128, 4], U32, tag=f"i_{name}")
            nc.sync.dma_start(isb, idx_d.ap())
            g = sb.tile([128, 4, 128], F32, tag=f"G_{name}")
            for j in range(4):
                nc.gpsimd.indirect_dma_start(
                    out=g[:, j, :], out_offset=None, in_=queue_d.ap(),
                    in_offset=IndirectOffsetOnAxis(ap=isb[:, j:j + 1], axis=0),
                )
            gat[name] = g

        a_sb = sb.tile([128, 4], F32)
        b_sb = sb.tile([128, 4], F32)
        nc.sync.dma_start(a_sb, a_d.ap())
        nc.sync.dma_start(b_sb, bb_d.ap())

        def mixdot(w_sb, x, y, out_d):
            """out = dot(normalize(w*x + (1-w)*y), oq_rep) / T  per (p, slot)."""
            winv = sb.tile([128, 4], F32, tag="winv")
            nc.vector.tensor_scalar(
                out=winv, in0=w_sb, scalar1=-1.0, scalar2=1.0,
                op0=ALU.mult, op1=ALU.add,
            )
            mix = sb.tile([128, 4, 128], F32, tag="mix")
            t2 = sb.tile([128, 4, 128], F32, tag="t2")
            w_b = w_sb[:, :].to_broadcast([128, 4, 128])
            winv_b = winv[:, :].to_broadcast([128, 4, 128])
            nc.vector.tensor_tensor(out=mix, in0=x, in1=w_b, op=ALU.mult)
            nc.vector.tensor_tensor(out=t2, in0=y, in1=winv_b, op=ALU.mult)
            nc.vector.tensor_tensor(out=mix, in0=mix, in1=t2, op=ALU.add)
            sq = sb.tile([128, 4, 128], F32, tag="sq")
            nc.vector.tensor_tensor(out=sq, in0=mix, in1=mix, op=ALU.mult)
            ss = sb.tile([128, 4], F32, tag="ss")
            nc.vector.tensor_reduce(out=ss, in_=sq, axis=AX.X, op=ALU.add)
            rt = sb.tile([128, 4], F32, tag="rt")
            nc.scalar.activation(rt, ss, AF.Sqrt)
            inv0 = sb.tile([128, 4], F32, tag="inv0")
            nc.vector.reciprocal(inv0, rt)
            # Newton polish
            t0 = sb.tile([128, 4], F32, tag="t0")
            nc.vector.tensor_tensor(out=t0, in0=inv0, in1=inv0, op=ALU.mult)
            nc.vector.tensor_tensor(out=t0, in0=t0, in1=ss, op=ALU.mult)
            nc.vector.tensor_scalar(
                out=t0, in0=t0, scalar1=-0.5, scalar2=1.5,
                op0=ALU.mult, op1=ALU.add,
            )
            inv1 = sb.tile([128, 4], F32, tag="inv1")
            nc.vector.tensor_tensor(out=inv1, in0=inv0, in1=t0, op=ALU.mult)
            dd3 = sb.tile([128, 4, 128], F32, tag="dd3")
            nc.vector.tensor_tensor(out=dd3, in0=mix, in1=oq_rep, op=ALU.mult)
            dd = sb.tile([128, 4], F32, tag="dd")
            nc.vector.tensor_reduce(out=dd, in_=dd3, axis=AX.X, op=ALU.add)
            nc.vector.tensor_tensor(out=dd, in0=dd, in1=inv1, op=ALU.mult)
            e_sb = sb.tile([128, 4], F32, tag="e_sb")
            nc.vector.tensor_scalar_mul(e_sb, dd, 1.0 / T)
            nc.sync.dma_start(out_d.ap(), e_sb)

        # type 1: alpha*Q[g1a] + (1-alpha)*Q[g1b]
        mixdot(a_sb, gat["g1a"], gat["g1b"], e1_d)
        # type 2: b*out_q + (1-b)*Q[g2]
        mixdot(b_sb, oq_rep, gat["g2"], e2_d)

    nc.compile()
    return nc, ["e1", "e2"]


# --------------------------------------------------------------------------
# Execution backends
# --------------------------------------------------------------------------

_CACHE = {}
LAST_STATS = {}


def _get(builder, key):
    if key not in _CACHE:
        _CACHE[key] = builder()
    return _CACHE[key]


def _run(nc, out_names, in_maps, label="a"):
    if os.environ.get("BASS_MOCO_BACKEND", "hw") == "sim":
        from concourse.bass_interp import CoreSim
        results = []
        for m in in_maps:
            sim = CoreSim(nc, trace=False)
            for name, arr in m.items():
                sim.tensor(name)[:] = arr
            sim.simulate(check_with_hw=False)
            results.append({t: np.array(sim.tensor(t)) for t in out_names})
        return results
    import time
    from concourse.bass_utils import run_bass_kernel_spmd
    trace = os.environ.get("BASS_MOCO_TRACE", "0") == "1"
    t0 = time.time()
    res = run_bass_kernel_spmd(
        nc, in_maps, core_ids=list(range(len(in_maps))), trace=trace,
    )
    wall = time.time() - t0
    LAST_STATS[label] = {"exec_time_ns": res.exec_time_ns, "wall_s": wall}
    return res.results


# --------------------------------------------------------------------------
# Host-side glue
# --------------------------------------------------------------------------

def _slotify(arr_rows):
    """[32, 16] -> [128, 4] slot layout: slot(p=g*32+q, j) = (q, s=g*4+j)."""
    return np.ascontiguousarray(
        arr_rows.reshape(32, 4, 4).transpose(1, 0, 2).reshape(128, 4)
    )


def _unslotify(arr_slot):
    """[128, 4] -> [32, 16]."""
    return np.ascontiguousarray(
        arr_slot.reshape(4, 32, 4).transpose(1, 0, 2).reshape(32, 16)
    )


def kernel(q, k, W_q, W_k, queue, alpha, beta, idxs1a, idxs1b, idxs2, step):
    step = int(step)
    q = np.ascontiguousarray(np.asarray(q, dtype=np.float32))
    k = np.ascontiguousarray(np.asarray(k, dtype=np.float32))
    W_q = np.ascontiguousarray(np.asarray(W_q, dtype=np.float32))
    W_k = np.ascontiguousarray(np.asarray(W_k, dtype=np.float32))
    queue = np.ascontiguousarray(np.asarray(queue, dtype=np.float32))

    nc_a, outs_a = _get(_build_launch_a, "a")
    in_maps = [
        {
            "q": q, "k": k, "wq": W_q, "wk": W_k,
            "qsh": np.ascontiguousarray(queue[c * KSH:(c + 1) * KSH]),
        }
        for c in range(NCORES)
    ]
    res_a = _run(nc_a, outs_a, in_maps, label="a")

    L = np.concatenate([r["L"] for r in res_a], axis=1)          # [256, 65536]
    pos = res_a[0]["pos"][0]                                     # [256]
    outq = np.ascontiguousarray(res_a[0]["outqT"].T)             # [256, 128]

    # chunk-level candidates: global chunk ids + device chunk maxima
    allc = np.concatenate(
        [c * NCHUNK + res_a[c]["candc"].astype(np.int64) for c in range(NCORES)],
        axis=1,
    )                                                            # [256, 320]
    allm = np.concatenate(
        [res_a[c]["candm"] for c in range(NCORES)], axis=1
    )                                                            # [256, 320]

    # top-44 chunks per row by device chunk-max (true top-32 elements are
    # guaranteed inside; 12 chunks of slack covers device-value error)
    NCH_KEEP = 44
    keep = np.argpartition(-allm, NCH_KEEP, axis=1)[:, :NCH_KEEP]
    chunks = np.take_along_axis(allc, keep, axis=1)              # [256, 44]
    cols = (chunks[:, :, None] * CHUNK
            + np.arange(CHUNK)[None, None, :]).reshape(B, -1)    # [256, 704]

    # host fp32 scoring of candidate columns, then fp64 re-rank of top-64
    # (fp64 ordering reproduces the fp32 jax reference ordering).
    qc = queue[cols]                                             # [256, 704, 128]
    vals32 = np.einsum("bd,bkd->bk", outq, qc)
    top64 = np.argpartition(-vals32, 64, axis=1)[:, :64]
    cols64 = np.take_along_axis(cols, top64, axis=1)
    vals64 = np.einsum(
        "bd,bkd->bk", outq.astype(np.float64),
        queue[cols64].astype(np.float64),
    )
    gidx = np.empty((B, N_HARD), dtype=np.int64)
    for r in range(B):
        order = np.lexsort((cols64[r], -vals64[r]))[:N_HARD]
        gidx[r] = cols64[r][order]

    parts = [pos[:, None].astype(np.float32), L]

    if step > START1 or step > START2:
        idxs1a = np.asarray(idxs1a); idxs1b = np.asarray(idxs1b)
        idxs2 = np.asarray(idxs2)
        alpha_s = np.asarray(alpha, dtype=np.float32)[:, :, 0]   # [256, 16]
        beta_s = np.asarray(beta, dtype=np.float32)[:, :, 0]
        bhalf = (beta_s * np.float32(BETA_HARD)).astype(np.float32)
        rows_idx = np.arange(B)[:, None]
        g1a = gidx[rows_idx, idxs1a].astype(np.uint32)           # [256, 16]
        g1b = gidx[rows_idx, idxs1b].astype(np.uint32)
        g2 = gidx[rows_idx, idxs2].astype(np.uint32)

        nc_b, outs_b = _get(_build_launch_b, "b")
        in_maps_b = []
        for c in range(NCORES):
            rs = slice(c * 32, (c + 1) * 32)
            # compact per-core row table: ship only the referenced queue rows
            need = np.concatenate(
                [g1a[rs].ravel(), g1b[rs].ravel(), g2[rs].ravel()]
            ).astype(np.int64)
            uniq = np.unique(need)
            tab = np.zeros((TAB_B, D), dtype=np.float32)
            tab[: len(uniq)] = queue[uniq]
            remap = {v: i for i, v in enumerate(uniq)}
            loc = np.vectorize(remap.__getitem__)
            in_maps_b.append({
                "tab": tab,
                "oq": np.ascontiguousarray(outq[rs]),
                "i1a": _slotify(loc(g1a[rs].astype(np.int64))).astype(np.uint32),
                "i1b": _slotify(loc(g1b[rs].astype(np.int64))).astype(np.uint32),
                "i2": _slotify(loc(g2[rs].astype(np.int64))).astype(np.uint32),
                "alpha": _slotify(alpha_s[rs]),
                "bhalf": _slotify(bhalf[rs]),
            })
        res_b = _run(nc_b, outs_b, in_maps_b, label="b")
        e1 = np.concatenate([_unslotify(r["e1"]) for r in res_b], axis=0)
        e2 = np.concatenate([_unslotify(r["e2"]) for r in res_b], axis=0)
        if step > START1:
            parts.append(e1.astype(np.float32))
        if step > START2:
            parts.append(e2.astype(np.float32))

    logits = np.concatenate(parts, axis=1)
    labels = np.zeros((B,), dtype=np.int32)
    return logits, labels
